# revision 1
# baseline (speedup 1.0000x reference)
# Depthwise causal conv1d (B=8, T=4096, C=1024, K=4, dilation=1) on 8 TRN2
# NeuronCores.
#
# Math: y[b, t, c] = sum_{j=0..3} weight[c, 3-j] * x[b, t-j, c]   (x[t<0] = 0)
#
# Strategy:
#   - Shard batch: core b handles x[b] (one full (T, C) slice).
#   - Host transposes each shard to (C, T) so the time axis is contiguous in
#     DRAM and lands on the SBUF free dimension; channels land on partitions.
#   - On-chip: for each 128-channel block, one [128, T+3] SBUF tile (3-col
#     zero halo at the left edge).  The 4 taps are applied by the TensorEngine
#     as 4 accumulating matmuls with a per-block *diagonal* weight matrix
#     lhsT = diag(w[cblock, 3-j]) against time-shifted rhs slices; PSUM does
#     the 4-tap accumulation for free.  fp32r keeps the PE at 1 cycle/row.
#   - DVE/ACT alternate on PSUM->SBUF copies; HWDGE DMAs move 2MB rows.
#   - Host transposes results back and stacks to (B, T, C).

import numpy as np

B, T, C, K = 8, 4096, 1024, 4
N_CORES = 8
P = 128  # SBUF partitions
NSUB = 512  # matmul free-dim (one fp32 PSUM bank)
HALO = 4  # leading zero columns (causal left pad), shipped from host

_CACHE = {}


def _build_nc(t_len=T, n_ch=C, mode="f32r"):
    import concourse.mybir as mybir
    import concourse.tile as tile
    from concourse import bacc
    from concourse.masks import make_identity

    f32 = mybir.dt.float32
    if mode == "f32r":
        cdt = mybir.dt.float32r
    elif mode == "bf16":
        cdt = mybir.dt.bfloat16
    else:
        cdt = f32
    ncb = n_ch // P  # channel blocks
    nsub = t_len // NSUB  # time sub-blocks per channel block

    # Bacc (not raw Bass): its compile() pass legalizes multi-wait sync into
    # event-semaphore instructions (TRN2 allows 1 wait per instruction).
    nc = bacc.Bacc(None)
    # x is declared with the compute dtype; for f32r this is a bit-identical
    # view of f32, for bf16 the (SWDGE) DMA casts inline.
    x_dt = cdt if mode == "f32r" else f32
    x = nc.declare_dram_parameter("x", [n_ch, t_len + HALO], x_dt, isOutput=False)
    # w_sb[p, cb*K + jj] = weight[cb*128 + p, jj]; diagonal lhsT blocks are
    # built on-chip (identity x per-partition scalar) to avoid a 2MB DMA.
    w = nc.declare_dram_parameter("w", [P, ncb * K], f32, isOutput=False)
    y = nc.declare_dram_parameter("y", [n_ch, t_len], f32, isOutput=True)

    # Each 128-channel block's time axis is processed as two half-rows of
    # t_len/2, each its own SBUF tile, so loads/stores move 1MB grains and
    # stores can start after half the block's PSUM copies.
    half = t_len // 2
    hsub = half // NSUB  # sub-blocks per half

    with tile.TileContext(nc) as tc:
        with (
            tc.tile_pool(name="const", bufs=1) as cpool,
            tc.tile_pool(name="xin", bufs=6) as xpool,
            tc.tile_pool(name="yout", bufs=4) as ypool,
            tc.tile_pool(name="ps", bufs=8, space="PSUM") as pspool,
        ):
            w_sb = cpool.tile([P, ncb * K], f32)
            nc.sync.dma_start(out=w_sb[:, :], in_=w[:, :])
            ident = cpool.tile([P, P], f32)
            make_identity(nc, ident)
            # wdiag[(cb, j)] holds diag(weight[cb*128 + p, K-1-j]).  One
            # tile per block: Tile tracks deps per tile, so the first
            # matmul only waits for its own diag, not all 32 builder ops.
            wdiag = {}
            for cb in range(ncb):
                for j in range(K):
                    col = cb * K + (K - 1 - j)
                    wd = cpool.tile([P, P], cdt, tag=f"wd_{cb}_{j}")
                    nc.vector.tensor_scalar_mul(
                        out=wd[:, :],
                        in0=ident[:, :],
                        scalar1=w_sb[:, col : col + 1],
                    )
                    wdiag[(cb, j)] = wd

            xdma = nc.gpsimd if mode == "bf16" else nc.sync
            for cb in range(ncb):
                rows = slice(cb * P, (cb + 1) * P)
                for h in range(2):
                    # half h covers t in [h*half, (h+1)*half); each x tile
                    # has HALO extra leading cols (zero pad for h=0, shipped
                    # by the host; overlap re-load of the previous 4 cols
                    # otherwise) so fp32r matmuls stay at N=512 any offset.
                    # The very first half-block is loaded as four 512-col
                    # piece-tiles so the PE starts after ~260KB, not 1MB.
                    first = cb == 0 and h == 0
                    if first:
                        xts = []
                        for m in range(hsub):
                            xp = xpool.tile([P, NSUB + HALO], cdt, tag="xhead")
                            xdma.dma_start(
                                out=xp[:, :],
                                in_=x[rows, NSUB * m : NSUB * (m + 1) + HALO],
                            )
                            xts.append(xp)
                    else:
                        xt = xpool.tile([P, half + HALO], cdt)
                        xdma.dma_start(
                            out=xt[:, :],
                            in_=x[rows, h * half : (h + 1) * half + HALO],
                        )
                    yt = ypool.tile([P, half], f32)
                    for m in range(hsub):
                        ps = pspool.tile([P, NSUB], f32)
                        for j in range(K):
                            # y[:, t] += diag(w[:, K-1-j]) @ x[:, t - j]
                            lhsT = wdiag[(cb, j)][:, :]
                            if first:
                                rhs = xts[m][:, HALO - j : HALO - j + NSUB]
                            else:
                                off = NSUB * m + HALO - j
                                rhs = xt[:, off : off + NSUB]
                            nc.tensor.matmul(
                                ps[:, :], lhsT, rhs,
                                start=(j == 0), stop=(j == K - 1),
                            )
                        dst = yt[:, NSUB * m : NSUB * (m + 1)]
                        if m % 2 == 0:
                            nc.vector.tensor_copy(dst, ps[:, :])
                        else:
                            nc.scalar.copy(dst, ps[:, :])
                    # Stores go out on the ACT HWDGE ring (nc.scalar) so they
                    # don't head-of-line-block the x loads on the SP ring.
                    nc.scalar.dma_start(
                        out=y[rows, h * half : (h + 1) * half], in_=yt[:, :]
                    )
    return nc


MODE = "f32r"  # compute dtype for the PE: "f32r" (2e-4 err) or "bf16" (faster)


def _get_nc():
    if "nc" not in _CACHE:
        nc = _build_nc(mode=MODE)
        # Bacc.finalize() runs compile(): moves matmul waits to ldweights,
        # splits multi-wait sync into event-sem instructions, allocates regs.
        nc.finalize()
        _CACHE["nc"] = nc
    return _CACHE["nc"]


def _pack_weight(weight):
    # w_sb[p, cb*K + jj] = weight[cb*P + p, jj]
    w = np.asarray(weight, dtype=np.float32)
    ncb = C // P
    return np.ascontiguousarray(
        w.reshape(ncb, P, K).transpose(1, 0, 2).reshape(P, ncb * K)
    )


LAST_RESULT = None


def kernel(x, weight):
    global LAST_RESULT
    from concourse.bass_utils import run_bass_kernel_spmd

    x = np.asarray(x, dtype=np.float32)
    w_sb = _pack_weight(weight)
    nc = _get_nc()

    in_maps = []
    for b in range(N_CORES):
        xt = np.zeros((C, T + HALO), dtype=np.float32)
        xt[:, HALO:] = x[b].T
        in_maps.append({"x": xt, "w": w_sb})
    res = run_bass_kernel_spmd(nc, in_maps, list(range(N_CORES)))
    LAST_RESULT = res

    y = np.empty((B, T, C), dtype=np.float32)
    for b in range(N_CORES):
        y[b] = res.results[b]["y"].T
    return y



# revision 2
# speedup vs baseline: 1.1908x; 1.1908x over previous
# Depthwise causal conv1d (B=8, T=4096, C=1024, K=4, dilation=1) on 8 TRN2
# NeuronCores.
#
# Math: y[b, t, c] = sum_{j=0..3} weight[c, 3-j] * x[b, t-j, c]   (x[t<0] = 0)
#
# Strategy (v2 — bf16 I/O):
#   - The kernel is HBM-bound (~358 GB/s per core).  All DRAM I/O is bf16:
#     the host rounds x to bf16 and upcasts y afterwards, halving traffic vs
#     f32 (16.8 MB/core -> ~47 us roofline; tolerance is 2e-2, bf16 ~6e-3).
#   - Shard batch: core b handles x[b] (one full (T, C) slice), host
#     transposes to (C, T) so time is contiguous and channels sit on
#     partitions.
#   - At bf16 the PE alone (4 taps x 4096 cols x 8 blocks = 131k cycles
#     ~ 55 us) would sit above the DMA floor, so compute is split:
#       * channel blocks 0..5: TensorEngine — 4 accumulating matmuls per
#         512-col sub-block against diagonal lhsT = diag(w[cblock, 3-j]);
#         PSUM does the 4-tap sum; ACT copies PSUM->SBUF (bf16 cast).
#       * channel blocks 6..7: DVE — tensor_scalar_mul for the first tap,
#         then 3 fused scalar_tensor_tensor MACs (out = x*w + out), all
#         bf16 SBUF->SBUF; block 7's leading mul runs on ACT instead.
#     Predicted busy: PE ~41 us, DVE ~28 us, ACT ~21 us — all under DMA.
#   - Loads ride the SP HWDGE ring (nc.sync), stores the ACT ring
#     (nc.scalar).

import numpy as np
import ml_dtypes

B, T, C, K = 8, 4096, 1024, 4
N_CORES = 8
P = 128  # SBUF partitions
NSUB = 512  # matmul free-dim (one fp32 PSUM bank)
HALO = 4  # leading zero columns (causal left pad), shipped from host
NCB = C // P  # channel blocks per core
PE_CB = 6  # channel blocks computed on the TensorEngine; rest on DVE/ACT

_CACHE = {}


def _build_nc():
    import concourse.mybir as mybir
    import concourse.tile as tile
    from concourse import bacc
    from concourse.masks import make_identity

    f32 = mybir.dt.float32
    bf16 = mybir.dt.bfloat16
    mult = mybir.AluOpType.mult
    addop = mybir.AluOpType.add

    half = T // 2
    hsub = half // NSUB  # sub-blocks per half

    # Bacc (not raw Bass): its compile() pass legalizes multi-wait sync into
    # event-semaphore instructions (TRN2 allows 1 wait per instruction).
    nc = bacc.Bacc(None)
    x = nc.declare_dram_parameter("x", [C, T + HALO], bf16, isOutput=False)
    # w_sb[p, cb*K + jj] = weight[cb*128 + p, jj] (f32; tiny)
    w = nc.declare_dram_parameter("w", [P, NCB * K], f32, isOutput=False)
    y = nc.declare_dram_parameter("y", [C, T], bf16, isOutput=True)

    with tile.TileContext(nc) as tc:
        with (
            tc.tile_pool(name="const", bufs=1) as cpool,
            tc.tile_pool(name="xin", bufs=6) as xpool,
            tc.tile_pool(name="yout", bufs=4) as ypool,
            tc.tile_pool(name="ps", bufs=8, space="PSUM") as pspool,
        ):
            w_sb = cpool.tile([P, NCB * K], f32)
            nc.sync.dma_start(out=w_sb[:, :], in_=w[:, :])
            ident = cpool.tile([P, P], f32)
            make_identity(nc, ident)
            # wdiag[(cb, j)] holds diag(weight[cb*128 + p, K-1-j]) in bf16.
            # One tile per block: Tile tracks deps per tile, so the first
            # matmul only waits for its own diag, not all builder ops.
            wdiag = {}
            for cb in range(PE_CB):
                for j in range(K):
                    col = cb * K + (K - 1 - j)
                    wd = cpool.tile([P, P], bf16, tag=f"wd_{cb}_{j}")
                    nc.vector.tensor_scalar_mul(
                        out=wd[:, :],
                        in0=ident[:, :],
                        scalar1=w_sb[:, col : col + 1],
                    )
                    wdiag[(cb, j)] = wd

            for cb in range(NCB):
                rows = slice(cb * P, (cb + 1) * P)
                pe_path = cb < PE_CB
                for h in range(2):
                    # half h covers t in [h*half, (h+1)*half); each x tile
                    # has HALO extra leading cols (zero pad for h=0, shipped
                    # by the host; overlap re-load of the previous 4 cols
                    # otherwise).  The very first half-block is loaded as
                    # four 512-col piece-tiles so the PE starts after
                    # ~130KB, not 525KB.
                    first = cb == 0 and h == 0
                    if first:
                        xts = []
                        for m in range(hsub):
                            xp = xpool.tile([P, NSUB + HALO], bf16, tag="xhead")
                            nc.sync.dma_start(
                                out=xp[:, :],
                                in_=x[rows, NSUB * m : NSUB * (m + 1) + HALO],
                            )
                            xts.append(xp)
                    else:
                        xt = xpool.tile([P, half + HALO], bf16)
                        nc.sync.dma_start(
                            out=xt[:, :],
                            in_=x[rows, h * half : (h + 1) * half + HALO],
                        )
                    yt = ypool.tile([P, half], bf16)
                    for m in range(hsub):

                        def xs(j):
                            # x[:, t - j] for t in this sub-block
                            if first:
                                return xts[m][:, HALO - j : HALO - j + NSUB]
                            off = NSUB * m + HALO - j
                            return xt[:, off : off + NSUB]

                        dst = yt[:, NSUB * m : NSUB * (m + 1)]
                        if pe_path:
                            ps = pspool.tile([P, NSUB], f32)
                            for j in range(K):
                                # y[:, t] += diag(w[:, K-1-j]) @ x[:, t - j]
                                nc.tensor.matmul(
                                    ps[:, :], wdiag[(cb, j)][:, :], xs(j),
                                    start=(j == 0), stop=(j == K - 1),
                                )
                            # All PSUM->SBUF copies (with bf16 cast) on ACT;
                            # DVE is busy with the vector-path blocks.
                            nc.scalar.copy(dst, ps[:, :])
                        else:
                            # dst = w[:,0]*x(t-3); dst = w[:,j']*x(t-j)+dst
                            c3 = cb * K  # column of weight[:, K-1-3=0]
                            if cb == NCB - 1:
                                nc.scalar.mul(dst, xs(3), w_sb[:, c3 : c3 + 1])
                            else:
                                nc.vector.tensor_scalar_mul(
                                    out=dst, in0=xs(3),
                                    scalar1=w_sb[:, c3 : c3 + 1],
                                )
                            for j in (2, 1, 0):
                                cj = cb * K + (K - 1 - j)
                                nc.vector.scalar_tensor_tensor(
                                    out=dst, in0=xs(j),
                                    scalar=w_sb[:, cj : cj + 1],
                                    in1=dst, op0=mult, op1=addop,
                                )
                    # Stores go out on the ACT HWDGE ring (nc.scalar) so they
                    # don't head-of-line-block the x loads on the SP ring.
                    nc.scalar.dma_start(
                        out=y[rows, h * half : (h + 1) * half], in_=yt[:, :]
                    )
    return nc


def _get_nc():
    if "nc" not in _CACHE:
        nc = _build_nc()
        # Bacc.finalize() runs compile(): moves matmul waits to ldweights,
        # splits multi-wait sync into event-sem instructions, allocates regs.
        nc.finalize()
        _CACHE["nc"] = nc
    return _CACHE["nc"]


def _to_bf16(a):
    # Fast round-to-nearest-even f32 -> bf16 via integer ops (no NaN/Inf in
    # this workload).  ml_dtypes astype is much slower.
    u = np.ascontiguousarray(a, dtype=np.float32).view(np.uint32)
    r = ((u + 0x7FFF + ((u >> 16) & 1)) >> 16).astype(np.uint16)
    return r.view(ml_dtypes.bfloat16)


def _from_bf16(a):
    u = np.asarray(a).view(np.uint16).astype(np.uint32) << 16
    return u.view(np.float32)


def _pack_weight(weight):
    # w_sb[p, cb*K + jj] = weight[cb*P + p, jj]
    w = np.asarray(weight, dtype=np.float32)
    return np.ascontiguousarray(
        w.reshape(NCB, P, K).transpose(1, 0, 2).reshape(P, NCB * K)
    )


def _make_in_maps(x, weight):
    x = np.asarray(x, dtype=np.float32)
    w_sb = _pack_weight(weight)
    in_maps = []
    for b in range(N_CORES):
        xt = np.zeros((C, T + HALO), dtype=ml_dtypes.bfloat16)
        xt[:, HALO:] = _to_bf16(x[b].T)
        in_maps.append({"x": xt, "w": w_sb})
    return in_maps


LAST_RESULT = None


def kernel(x, weight):
    global LAST_RESULT
    from concourse.bass_utils import run_bass_kernel_spmd

    nc = _get_nc()
    in_maps = _make_in_maps(x, weight)
    res = run_bass_kernel_spmd(nc, in_maps, list(range(N_CORES)))
    LAST_RESULT = res

    y = np.empty((B, T, C), dtype=np.float32)
    for b in range(N_CORES):
        y[b] = _from_bf16(res.results[b]["y"]).T
    return y


# revision 5
# speedup vs baseline: 1.5305x; 1.2852x over previous
# Depthwise causal conv1d (B=8, T=4096, C=1024, K=4, dilation=1) on 8 TRN2
# NeuronCores.
#
# Math: y[b, t, c] = sum_{j=0..3} weight[c, 3-j] * x[b, t-j, c]   (x[t<0] = 0)
#
# Strategy (v2.1 — bf16 I/O, balanced engines):
#   - HBM-bound problem (~358 GB/s per core).  All DRAM I/O is bf16: the
#     host rounds x to bf16 and upcasts y afterwards, halving traffic vs f32
#     (16.8 MB/core total -> ~50 us DMA floor; tolerance 2e-2, bf16 ~5e-3).
#   - Shard batch: core b handles x[b]; host transposes to (C, T) so time is
#     contiguous and channels sit on partitions.
#   - PE alone at bf16 (4 taps x 4096 cols x 8 blocks ~ 55 us) would top the
#     DMA floor, so channel block 7 runs on DVE/ACT instead:
#       ACT: t1 = w3*x(t-3), t2 = w1*x(t-1)   (activation scale = per-chan w)
#       DVE: t1 += w2*x(t-2), t2 += w0*x(t)   (fused scalar_tensor_tensor)
#       DVE: y = t1 + t2                      (tensor_tensor, bf16 2x mode)
#     Chain work is interleaved with the PE blocks in program order and its
#     x row is loaded first so it overlaps the whole span.
#   - PSUM->SBUF drain is the other big tax (~1 col/cycle, PSUM reads are
#     1x): one [128, 2048] copy per half (not 4x512) to amortize fixed
#     costs, alternating DVE/ACT.
#   - All x tiles stay resident (8.4 MB < SBUF); loads prequeued on the SP
#     HWDGE ring in consumption order, stores ride the ACT ring.

import numpy as np
import ml_dtypes

B, T, C, K = 8, 4096, 1024, 4
N_CORES = 8
P = 128  # SBUF partitions
NSUB = 512  # matmul free-dim (one fp32 PSUM bank)
HALF = T // 2
HSUB = HALF // NSUB
HALO = 4  # leading zero columns (causal left pad), shipped from host
NCB = C // P  # channel blocks per core
PE_CB = 7  # channel blocks computed on the TensorEngine; rest on DVE/ACT

_CACHE = {}


def _build_nc():
    import concourse.mybir as mybir
    import concourse.tile as tile
    from concourse import bacc
    from concourse.masks import make_identity

    f32 = mybir.dt.float32
    bf16 = mybir.dt.bfloat16
    mult = mybir.AluOpType.mult
    addop = mybir.AluOpType.add

    nc = bacc.Bacc(None)
    x = nc.declare_dram_parameter("x", [C, T + HALO], bf16, isOutput=False)
    # w_sb[p, cb*K + jj] = weight[cb*128 + p, jj] (f32; tiny)
    w = nc.declare_dram_parameter("w", [P, NCB * K], f32, isOutput=False)
    y = nc.declare_dram_parameter("y", [C, T], bf16, isOutput=True)

    vcbs = list(range(PE_CB, NCB))  # vector-path channel blocks

    with tile.TileContext(nc) as tc:
        with (
            tc.tile_pool(name="const", bufs=1) as cpool,
            tc.tile_pool(name="xin", bufs=1) as xpool,
            tc.tile_pool(name="yv", bufs=1) as yvpool,
            tc.tile_pool(name="yout", bufs=4) as ypool,
            tc.tile_pool(name="tmp", bufs=4) as tpool,
            tc.tile_pool(name="ps", bufs=2, space="PSUM") as pspool,
        ):
            w_sb = cpool.tile([P, NCB * K], f32)
            nc.sync.dma_start(out=w_sb[:, :], in_=w[:, :])
            ident = cpool.tile([P, P], bf16)
            make_identity(nc, ident)

            # ---- all x loads, prequeued in consumption order ----
            # (vector-path rows first after the PE head so chains can run
            # through the whole span; everything stays resident in SBUF)
            xt = {}  # (cb, h) -> tile; (0, 0) -> list of piece tiles
            head = []
            for m in range(HSUB):
                xp = xpool.tile([P, NSUB + HALO], bf16, name=f"xhead{m}", tag=f"xhead{m}")
                nc.sync.dma_start(
                    out=xp[:, :], in_=x[0:P, NSUB * m : NSUB * (m + 1) + HALO]
                )
                head.append(xp)
            xt[(0, 0)] = head
            order = [(vcb, h) for vcb in vcbs for h in range(2)]
            order += [(0, 1)]
            order += [(cb, h) for cb in range(1, PE_CB) for h in range(2)]
            for cb, h in order:
                t = xpool.tile([P, HALF + HALO], bf16, name=f"x_{cb}_{h}", tag=f"x_{cb}_{h}")
                nc.sync.dma_start(
                    out=t[:, :],
                    in_=x[cb * P : (cb + 1) * P, h * HALF : (h + 1) * HALF + HALO],
                )
                xt[(cb, h)] = t

            def xs(cb, h, m, j):
                # x[:, t - j] slice for sub-block m of half h
                if cb == 0 and h == 0:
                    return xt[(0, 0)][m][:, HALO - j : HALO - j + NSUB]
                off = NSUB * m + HALO - j
                return xt[(cb, h)][:, off : off + NSUB]

            # wdiag[(cb, j)] = diag(weight[cb*128 + p, K-1-j]) in bf16;
            # built on DVE from the bf16 identity (4x tensor_scalar mode).
            wdiag = {}

            def build_wdiag(cb):
                for j in range(K):
                    col = cb * K + (K - 1 - j)
                    wd = cpool.tile([P, P], bf16, tag=f"wd_{cb}_{j}")
                    nc.vector.tensor_scalar_mul(
                        out=wd[:, :],
                        in0=ident[:, :],
                        scalar1=w_sb[:, col : col + 1],
                    )
                    wdiag[(cb, j)] = wd

            build_wdiag(0)
            if PE_CB > 1:
                build_wdiag(1)

            # vector-path chain for one 512-col sub-block
            def emit_chain(vcb, v):
                h, m = divmod(v, HSUB)
                dst = ytv[(vcb, h)][:, NSUB * m : NSUB * (m + 1)]
                t1 = tpool.tile([P, NSUB], bf16, tag="t1")
                t2 = tpool.tile([P, NSUB], bf16, tag="t2")
                cw = lambda j: w_sb[:, vcb * K + (K - 1 - j) : vcb * K + (K - j)]
                nc.scalar.mul(t1[:, :], xs(vcb, h, m, 3), cw(3))
                nc.scalar.mul(t2[:, :], xs(vcb, h, m, 1), cw(1))
                nc.vector.scalar_tensor_tensor(
                    out=t1[:, :], in0=xs(vcb, h, m, 2), scalar=cw(2),
                    in1=t1[:, :], op0=mult, op1=addop,
                )
                nc.vector.scalar_tensor_tensor(
                    out=t2[:, :], in0=xs(vcb, h, m, 0), scalar=cw(0),
                    in1=t2[:, :], op0=mult, op1=addop,
                )
                nc.vector.tensor_add(dst, t1[:, :], t2[:, :])

            # y tiles for the vector-path blocks (written by chains, stored
            # once per half when all 4 sub-blocks are done)
            ytv = {}
            for vcb in vcbs:
                for h in range(2):
                    ytv[(vcb, h)] = yvpool.tile([P, HALF], bf16, name=f"yv{vcb}_{h}", tag=f"yv{vcb}_{h}")

            # ---- main loop: PE halves, with vector chains interleaved ----
            pe_halves = [(cb, h) for cb in range(PE_CB) for h in range(2)]
            nvec = 2 * HSUB * len(vcbs)
            chain_after = {}  # pe-half index -> list of (vcb, v)
            for i in range(nvec):
                k = min(len(pe_halves) - 1, (i * len(pe_halves)) // max(nvec, 1))
                chain_after.setdefault(k, []).append(
                    (vcbs[i // (2 * HSUB)], i % (2 * HSUB))
                )

            for k, (cb, h) in enumerate(pe_halves):
                rows = slice(cb * P, (cb + 1) * P)
                ps = pspool.tile([P, HALF], f32)
                for m in range(HSUB):
                    for j in range(K):
                        # y[:, t] += diag(w[:, K-1-j]) @ x[:, t - j]
                        nc.tensor.matmul(
                            ps[:, NSUB * m : NSUB * (m + 1)],
                            wdiag[(cb, j)][:, :],
                            xs(cb, h, m, j),
                            start=(j == 0),
                            stop=(j == K - 1),
                        )
                if h == 0 and cb + 2 < PE_CB:
                    build_wdiag(cb + 2)
                yt = ypool.tile([P, HALF], bf16)
                if k % 2 == 0:
                    nc.vector.tensor_copy(yt[:, :], ps[:, :])
                else:
                    nc.scalar.copy(yt[:, :], ps[:, :])
                nc.scalar.dma_start(
                    out=y[rows, h * HALF : (h + 1) * HALF], in_=yt[:, :]
                )
                for vcb, v in chain_after.get(k, []):
                    emit_chain(vcb, v)
                    if v % HSUB == HSUB - 1:  # half complete -> store it
                        vh = v // HSUB
                        nc.scalar.dma_start(
                            out=y[
                                vcb * P : (vcb + 1) * P,
                                vh * HALF : (vh + 1) * HALF,
                            ],
                            in_=ytv[(vcb, vh)][:, :],
                        )
    return nc


def _get_nc():
    if "nc" not in _CACHE:
        nc = _build_nc()
        nc.finalize()
        _CACHE["nc"] = nc
    return _CACHE["nc"]


def _to_bf16(a):
    # Fast round-to-nearest-even f32 -> bf16 via integer ops (no NaN/Inf in
    # this workload).  ml_dtypes astype is much slower.
    u = np.ascontiguousarray(a, dtype=np.float32).view(np.uint32)
    r = ((u + 0x7FFF + ((u >> 16) & 1)) >> 16).astype(np.uint16)
    return r.view(ml_dtypes.bfloat16)


def _from_bf16(a):
    u = np.asarray(a).view(np.uint16).astype(np.uint32) << 16
    return u.view(np.float32)


def _pack_weight(weight):
    # w_sb[p, cb*K + jj] = weight[cb*P + p, jj]
    w = np.asarray(weight, dtype=np.float32)
    return np.ascontiguousarray(
        w.reshape(NCB, P, K).transpose(1, 0, 2).reshape(P, NCB * K)
    )


def _make_in_maps(x, weight):
    x = np.asarray(x, dtype=np.float32)
    w_sb = _pack_weight(weight)
    in_maps = []
    for b in range(N_CORES):
        xt = np.zeros((C, T + HALO), dtype=ml_dtypes.bfloat16)
        xt[:, HALO:] = _to_bf16(x[b].T)
        in_maps.append({"x": xt, "w": w_sb})
    return in_maps


LAST_RESULT = None


def kernel(x, weight):
    global LAST_RESULT
    from concourse.bass_utils import run_bass_kernel_spmd

    nc = _get_nc()
    in_maps = _make_in_maps(x, weight)
    res = run_bass_kernel_spmd(nc, in_maps, list(range(N_CORES)))
    LAST_RESULT = res

    y = np.empty((B, T, C), dtype=np.float32)
    for b in range(N_CORES):
        y[b] = _from_bf16(res.results[b]["y"]).T
    return y
